# revision 24
# baseline (speedup 1.0000x reference)
"""Trainium2 Bass kernel for a causal attention block (B=2, T=2048, E=2048,
16 heads, head_dim=128, interleaved RoPE).

Sharding: data-parallel over batch (2) x tensor-parallel over heads (4 per
core) = 8 NeuronCores. Each core computes QKV projection for its 4 heads,
RoPE, causal SDPA, and a partial output projection (row-sharded W_out); the
host sums the 4 TP partials per batch element.

Single software-pipelined instruction stream (fp16 matmuls, fp32 PSUM):
  - warmup matmuls on a memset tile ramp the PE p-state while the first
    DMAs land.
  - segment g in 0..3 runs the QKV projection for token slab g (QT/KT
    transposed + RoPE'd on eviction, V natural) interleaved with SDPA for
    query tile g-1; causality means tile g-1 only needs slabs <= g-1, and
    the dependency-free projection matmuls keep the PE busy while ACT/DVE
    drain the softmax chain.
  - SDPA diagonal blocks are column-restricted (queries below the diagonal
    are never computed), the softmax denominator tree runs in fp16 (DVE 4x
    mode) + a ones-matmul partition reduction, and the output projection
    for query tiles 0..2 interleaves into the tq=3 SDPA round.
"""

import sys
from contextlib import ExitStack

sys.path.insert(0, "/opt/trn_rl_repo")

import numpy as np

import bass_rust
import concourse.bacc as bacc
import concourse.mybir as mybir
from concourse.alu_op_type import AluOpType
from concourse import tile
from concourse import bass_utils

B, T, E = 2, 2048, 2048
N_HEAD = 16
D = E // N_HEAD            # 128
THETA = 10000.0
N_CORES = 8
TP = 4                     # tensor-parallel degree (heads)
HPC = N_HEAD // TP         # heads per core = 4
FL = HPC * D               # local head width = 512
EC = E // 128              # 16 contraction chunks
TQ = 512                   # query tile (free dim)
NTQ = T // TQ              # 4
NTK = T // 128             # 16
N_WARM = 8                 # PE p-state warmup matmuls

F32 = mybir.dt.float32
F32R = mybir.dt.float32r
F16 = mybir.dt.float16
EXP = mybir.ActivationFunctionType.Exp
SCALE = 1.0 / np.sqrt(D)

_compiled = None
_last_in_maps = None


def _build():
    nc = bacc.Bacc("TRN2", target_bir_lowering=False)

    xT = nc.dram_tensor("xT", (NTQ, 128, EC * TQ), F16, kind="ExternalInput")
    wqk = nc.dram_tensor("wqk", (2 * HPC, 128, EC * 128), F16, kind="ExternalInput")
    wv = nc.dram_tensor("wv", (128, EC * FL), F16, kind="ExternalInput")
    wout = nc.dram_tensor("wout", (HPC, 128, E), F16, kind="ExternalInput")
    csx = nc.dram_tensor("csx", (128, T), F16, kind="ExternalInput")
    csx2 = nc.dram_tensor("csx2", (128, T), F16, kind="ExternalInput")
    mask1 = nc.dram_tensor("mask1", (128, 128), F16, kind="ExternalInput")
    ones_m = nc.dram_tensor("ones_m", (128, 128), F16, kind="ExternalInput")
    out = nc.dram_tensor("out", (T, E), F16, kind="ExternalOutput")

    with tile.TileContext(nc) as tc, nc.allow_low_precision(
        reason="fp16 matmul inputs / fp16 softmax stats are intentional"
    ), tc.tile_pool(name="const", bufs=1) as const, \
         tc.tile_pool(name="wo_p", bufs=1) as wo_p, \
         tc.tile_pool(name="qkt_p", bufs=1) as qkt_p, \
         tc.tile_pool(name="v_p", bufs=1) as v_p, \
         tc.tile_pool(name="yt_p", bufs=1) as yt_p, \
         tc.tile_pool(name="es_p", bufs=10) as es_p, \
         tc.tile_pool(name="pair_p", bufs=4) as pair_p, \
         tc.tile_pool(name="dn_p", bufs=2) as dn_p, \
         tc.tile_pool(name="mm_ps", bufs=2, space="PSUM") as mm_ps, \
         tc.tile_pool(name="sc_ps", bufs=3, space="PSUM") as sc_ps, \
         tc.tile_pool(name="y_ps", bufs=2, space="PSUM") as y_ps, \
         tc.tile_pool(name="aux_ps", bufs=1, space="PSUM") as aux_ps:

        # phase-1 pools: closed before the final projection segment to free
        # SBUF for the output-eviction pool (stack allocation is LIFO)
        p1_stack = ExitStack()
        wqk_p = p1_stack.enter_context(tc.tile_pool(name="wqk_p", bufs=1))
        wv_p = p1_stack.enter_context(tc.tile_pool(name="wv_p", bufs=1))
        xt_p = p1_stack.enter_context(tc.tile_pool(name="xt_p", bufs=2))
        rope_t = p1_stack.enter_context(tc.tile_pool(name="rope_t", bufs=2))

        cs_sb = const.tile([128, T], F16, tag="cs")    # [cos; sin]
        csd_sb = const.tile([128, T], F16, tag="csd")  # [sin; cos]
        mask_sb = const.tile([128, 128], F16, tag="mask")
        onem = const.tile([128, 128], F16, tag="onem")
        warm_sb = const.tile([128, TQ], F16, tag="warm")

        # wqk stored f8-major: one [128, EC*128] tile per 128-wide qk block so
        # the first projection unit is gated by 0.5MB of weights, not 4MB
        wqk_sb = [wqk_p.tile([128, EC * 128], F16, tag=f"wqk{f}", name=f"wqk_sb{f}")
                  for f in range(2 * HPC)]
        wv_all = wv_p.tile([128, EC * FL], F16, tag="wv")
        wo_sb = [wo_p.tile([128, E], F16, tag=f"wo{h}", name=f"wo_sb{h}")
                 for h in range(HPC)]

        # resident intermediates: QT/KT (transposed, de-interleaved, RoPE'd),
        # V (natural layout), normalized attention outputs
        qkt_sb = [[qkt_p.tile([128, TQ], F16, tag=f"qkt{f}_{t}", name=f"qkt_sb{f}_{t}")
                   for t in range(NTQ)] for f in range(2 * HPC)]
        v_sb = [v_p.tile([128, FL], F16, tag=f"v{t}", name=f"v_sb{t}")
                for t in range(NTK)]
        yt_sb = [[yt_p.tile([128, TQ], F16, tag=f"yt{h}_{t}", name=f"yt_sb{h}_{t}")
                  for t in range(NTQ)] for h in range(HPC)]

        # ---------------- DMA + warmup ----------------
        xt_sb = {}

        def dma_slab(t4):
            xt = xt_p.tile([128, EC * TQ], F16, tag="xt", name=f"xt_{t4}")
            nc.sync.dma_start(xt[:], xT[t4])
            xt_sb[t4] = xt

        # PE p-state warmup on a memset tile (no DMA dependency)
        nc.vector.memset(warm_sb[:], 1.0)

        # startup DMA: slab 0 split into per-chunk column DMAs in consumption
        # order on one queue so the first QK unit streams as chunks land
        # (subtile deps); weights flow in parallel on the other two queues
        xt0 = xt_p.tile([128, EC * TQ], F16, tag="xt", name="xt_0")
        xt_sb[0] = xt0

        for e in range(EC):
            nc.sync.dma_start(xt0[:, e * TQ:(e + 1) * TQ],
                              xT[0, :, e * TQ:(e + 1) * TQ])
        nc.sync.dma_start(cs_sb[:], csx[:])
        nc.sync.dma_start(csd_sb[:], csx2[:])
        # first two qk weight blocks chunked so matmul e can start as soon
        # as its 128KB lands (subtile deps) despite slow early DMA
        for c in range(4):
            nc.scalar.dma_start(wqk_sb[0][:, c * 512:(c + 1) * 512],
                                wqk[0, :, c * 512:(c + 1) * 512])
        for c in range(2):
            nc.scalar.dma_start(wqk_sb[1][:, c * 1024:(c + 1) * 1024],
                                wqk[1, :, c * 1024:(c + 1) * 1024])
        for f in range(2, 2 * HPC):
            nc.scalar.dma_start(wqk_sb[f][:], wqk[f])
        nc.gpsimd.dma_start(wv_all[:], wv[:])
        nc.gpsimd.dma_start(mask_sb[:], mask1[:])
        nc.gpsimd.dma_start(onem[:], ones_m[:])
        for h in range(HPC):
            nc.scalar.dma_start(wo_sb[h][:], wout[h])

        wps = aux_ps.tile([128, TQ], F32, tag="aux", name="warm_ps")
        for i in range(N_WARM):
            nc.tensor.matmul(wps[:], warm_sb[:, 0:128], warm_sb[:],
                             start=True, stop=True, skip_group_check=True)

        # ---------------- unit builders ----------------
        def qk_unit(t4, f8):
            def f():
                ps = mm_ps.tile([128, TQ], F32, tag="mm", name=f"qkps_{t4}_{f8}")
                for e in range(EC):
                    nc.tensor.matmul(
                        ps[:], wqk_sb[f8][:, e * 128:(e + 1) * 128],
                        xt_sb[t4][:, e * TQ:(e + 1) * TQ],
                        start=(e == 0), stop=(e == EC - 1),
                        skip_group_check=True,
                    )
                ts4 = slice(t4 * TQ, (t4 + 1) * TQ)
                dst = qkt_sb[f8][t4]
                qk16 = rope_t.tile([128, TQ], F16, tag="qk16")
                nc.scalar.copy(qk16[:], ps[:])
                t_a = rope_t.tile([64, TQ], F16, tag="ta")
                t_b = rope_t.tile([64, TQ], F16, tag="tb")
                nc.vector.tensor_tensor(t_a[:], qk16[0:64, :], cs_sb[0:64, ts4], op=AluOpType.mult)
                nc.vector.tensor_tensor(t_b[:], qk16[64:128, :], cs_sb[64:128, ts4], op=AluOpType.mult)
                nc.vector.tensor_tensor(dst[0:64, :], t_a[:], t_b[:], op=AluOpType.subtract)
                t_c = rope_t.tile([64, TQ], F16, tag="tc")
                t_d = rope_t.tile([64, TQ], F16, tag="td")
                nc.vector.tensor_tensor(t_c[:], qk16[0:64, :], csd_sb[0:64, ts4], op=AluOpType.mult)
                nc.vector.tensor_tensor(t_d[:], qk16[64:128, :], csd_sb[64:128, ts4], op=AluOpType.mult)
                nc.vector.tensor_tensor(dst[64:128, :], t_c[:], t_d[:], op=AluOpType.add)
            return (16 * 216, f)

        def v_unit(t4, i):
            def f():
                tk = 4 * t4 + i
                ps = mm_ps.tile([128, FL], F32, tag="mm", name=f"vps_{tk}")
                for e in range(EC):
                    nc.tensor.matmul(
                        ps[:], xt_sb[t4][:, e * TQ + i * 128:e * TQ + (i + 1) * 128],
                        wv_all[:, e * FL:(e + 1) * FL],
                        start=(e == 0), stop=(e == EC - 1),
                        skip_group_check=True,
                    )
                nc.scalar.copy(v_sb[tk][:], ps[:])
            return (16 * 216, f)

        def p1_units(t4):
            return [qk_unit(t4, f8) for f8 in range(2 * HPC)] + \
                   [v_unit(t4, i) for i in range(4)]

        def sdpa_units(tq):
            units = []
            head_units = []
            for h in range(HPC):
                st = {"fulls": [], "rag": [], "yps": None}
                nblk = 4 * tq + 4

                def blk(h, tq, tk, st):
                    def f():
                        r = tk - 4 * tq
                        diag = r >= 0
                        c0 = r * 128 if diag else 0
                        cr = slice(c0, TQ)
                        if tk == 0:
                            st["yps"] = y_ps.tile([128, TQ], F32, tag="y",
                                                  name=f"yps_{h}_{tq}")
                        yps = st["yps"]
                        sps = sc_ps.tile([128, TQ], F32, tag="sc",
                                         name=f"sps_{h}_{tq}_{tk}")
                        nc.tensor.matmul(
                            sps[:, cr],
                            qkt_sb[HPC + h][tk // 4][:, (tk % 4) * 128:(tk % 4 + 1) * 128],
                            qkt_sb[h][tq][:, cr],
                            start=True, stop=True, skip_group_check=True,
                        )
                        es = es_p.tile([128, TQ], F16, tag="es",
                                       name=f"es_{h}_{tq}_{tk}")
                        nc.scalar.activation(es[:, cr], sps[:, cr], EXP, scale=SCALE)
                        if diag:
                            nc.vector.tensor_tensor(
                                es[:, c0:c0 + 128], es[:, c0:c0 + 128],
                                mask_sb[:], op=AluOpType.mult,
                            )
                        nc.tensor.matmul(
                            yps[:, cr], v_sb[tk][:, h * 128:(h + 1) * 128],
                            es[:, cr],
                            start=(tk == 0), stop=(tk == 4 * tq + 3),
                            skip_group_check=True,
                        )
                        # denominator bookkeeping: full-width tiles pair up in
                        # fp16 (DVE 4x); ragged diagonals accumulate at the end
                        if not diag or r == 0:
                            st["fulls"].append(es)
                            if len(st["fulls"]) >= 2:
                                a = st["fulls"].pop(0)
                                b = st["fulls"].pop(0)
                                pr = pair_p.tile([128, TQ], F16, tag="pr",
                                                 name=f"pr_{h}_{tq}_{tk}")
                                nc.vector.tensor_tensor(pr[:], a[:], b[:],
                                                        op=AluOpType.add)
                                st["fulls"].append(pr)
                        else:
                            st["rag"].append((es, c0))
                    return (int(432 * (TQ - (max(tk - 4 * tq, 0)) * 128) / TQ), f)

                def denom(h, tq, st):
                    def f():
                        dacc = st["fulls"][0]
                        for es, c0 in st["rag"]:
                            nc.vector.tensor_tensor(
                                dacc[:, c0:], dacc[:, c0:], es[:, c0:],
                                op=AluOpType.add,
                            )
                        # ones[128,128] @ dacc = column sums replicated on all
                        # partitions: reduction + broadcast in one matmul
                        dbc = aux_ps.tile([128, TQ], F32, tag="aux",
                                          name=f"dbc_{h}_{tq}")
                        nc.tensor.matmul(dbc[:], onem[:], dacc[:],
                                         start=True, stop=True,
                                         skip_group_check=True)
                        rcp = dn_p.tile([128, TQ], F32, tag="rcp",
                                        name=f"rcp_{h}_{tq}")
                        nc.vector.reciprocal_approx_fast(rcp[:], dbc[:])
                        nc.vector.tensor_tensor(
                            yt_sb[h][tq][:], st["yps"][:], rcp[:],
                            op=AluOpType.mult,
                        )
                    return (300, f)

                head_units.append(
                    ([blk(h, tq, tk, st) for tk in range(nblk)],
                     denom(h, tq, st)))
            for h in range(HPC):
                blocks, dn = head_units[h]
                k = min(2, len(blocks))
                units.extend(blocks[:k])
                if h > 0:
                    units.append(head_units[h - 1][1])
                units.extend(blocks[k:])
            units.append(head_units[HPC - 1][1])
            return units

        pools = {}

        def proj_unit(tqb, nb):
            def f():
                ps = mm_ps.tile([128, TQ], F32, tag="mm", name=f"ops_{tqb}_{nb}")
                for h in range(HPC):
                    nc.tensor.matmul(
                        ps[:],
                        yt_sb[h][tqb // 4][:, (tqb % 4) * 128:(tqb % 4 + 1) * 128],
                        wo_sb[h][:, nb * TQ:(nb + 1) * TQ],
                        start=(h == 0), stop=(h == HPC - 1),
                        skip_group_check=True,
                    )
                osb = pools["o_ev"].tile([128, TQ], F16, tag="osb",
                                         name=f"osb_{tqb}_{nb}")
                if nb % 2 == 0:
                    nc.vector.tensor_copy(osb[:], ps[:])
                else:
                    nc.scalar.copy(osb[:], ps[:])
                nc.sync.dma_start(
                    out[tqb * 128:(tqb + 1) * 128, nb * TQ:(nb + 1) * TQ], osb[:]
                )
            return (4 * 216, f)

        # ---------------- weave + emit ----------------
        def weave(a, b, bias=2000):
            # a = filler stream (phase-1/proj), b = latency-sensitive stream
            # (sdpa); bias keeps b ahead so a covers the segment tail
            ta = sum(w for w, _ in a) or 1
            tb = sum(w for w, _ in b) or 1
            ca = cb = 0
            i = j = 0
            while i < len(a) or j < len(b):
                if j >= len(b) or (i < len(a) and ca * tb <= max(cb - bias, 0) * ta):
                    ca += a[i][0]
                    a[i][1]()
                    i += 1
                else:
                    cb += b[j][0]
                    b[j][1]()
                    j += 1

        for _, f in p1_units(0):
            f()
        for g in range(1, NTQ):
            dma_slab(g)
            weave(p1_units(g), sdpa_units(g - 1))
        p1_stack.close()
        with tc.tile_pool(name="o_ev", bufs=4) as o_ev:
            pools["o_ev"] = o_ev
            proj = [proj_unit(tqb, nb) for tqb in range(12) for nb in range(4)]
            weave(proj[:36], sdpa_units(NTQ - 1))
            for _, f in proj[36:]:
                f()
            for tqb in range(12, 16):
                for nb in range(4):
                    proj_unit(tqb, nb)[1]()

    nc.compile()
    return nc


def _host_tables():
    positions = np.arange(T, dtype=np.float64)
    inv_freq = 1.0 / (THETA ** (np.arange(0, D, 2, dtype=np.float64) / D))
    freqs = np.outer(positions, inv_freq)          # [T, 64]
    cs = np.concatenate([np.cos(freqs).T, np.sin(freqs).T]).astype(np.float16)   # [128, T]
    cs2 = np.concatenate([np.sin(freqs).T, np.cos(freqs).T]).astype(np.float16)  # swapped halves
    p = np.arange(128)[:, None]
    j = np.arange(128)[None, :]
    mask = (p <= j).astype(np.float16)             # [128, 128] lower-tri visibility
    return cs, cs2, mask


def kernel(x, W_qkv, W_out):
    global _compiled
    if _compiled is None:
        _compiled = _build()
    nc = _compiled

    x = np.ascontiguousarray(np.asarray(x, dtype=np.float32))
    W_qkv = np.asarray(W_qkv, dtype=np.float32)
    W_out = np.asarray(W_out, dtype=np.float32)

    cs, cs2, mask = _host_tables()
    ones_mm = np.ones((128, 128), np.float16)

    perm = np.concatenate([np.arange(0, D, 2), np.arange(1, D, 2)])  # de-interleave

    in_maps = []
    for c in range(N_CORES):
        b, tp = divmod(c, TP)
        heads = np.arange(tp * HPC, (tp + 1) * HPC)
        qk_cols = np.concatenate(
            [h * D + perm for h in heads] + [E + h * D + perm for h in heads]
        )
        v_cols = np.concatenate([2 * E + h * D + np.arange(D) for h in heads])
        # wqk f8-major: [f8, p, e*128+m]
        wqk_l = np.ascontiguousarray(
            W_qkv[:, qk_cols].reshape(EC, 128, 2 * HPC, 128)
            .transpose(2, 1, 0, 3).reshape(2 * HPC, 128, EC * 128)
        )
        # wv e-major along free dim: [p, e*FL+m]
        wv_l = np.ascontiguousarray(
            W_qkv[:, v_cols].reshape(EC, 128, FL)
            .transpose(1, 0, 2).reshape(128, EC * FL)
        )
        wout_l = np.ascontiguousarray(
            W_out.reshape(N_HEAD, D, E)[heads].reshape(HPC, 128, E)
        )
        # x slab: [t4, p, e*TQ+t]
        xt4 = np.ascontiguousarray(
            x[b].reshape(NTQ, TQ, EC, 128).transpose(0, 3, 2, 1)
            .reshape(NTQ, 128, EC * TQ)
        ).astype(np.float16)
        in_maps.append({
            "xT": xt4,
            "wqk": wqk_l.astype(np.float16),
            "wv": wv_l.astype(np.float16),
            "wout": wout_l.astype(np.float16),
            "csx": cs,
            "csx2": cs2,
            "mask1": mask,
            "ones_m": ones_mm,
        })

    global _last_in_maps
    _last_in_maps = in_maps
    res = bass_utils.run_bass_kernel_spmd(nc, in_maps, core_ids=list(range(N_CORES)))
    out = np.zeros((B, T, E), dtype=np.float32)
    for c in range(N_CORES):
        out[c // TP] += res.results[c]["out"]
    return out


# revision 25
# speedup vs baseline: 1.0281x; 1.0281x over previous
"""Trainium2 Bass kernel for a causal attention block (B=2, T=2048, E=2048,
16 heads, head_dim=128, interleaved RoPE).

Sharding: data-parallel over batch (2) x tensor-parallel over heads (4 per
core) = 8 NeuronCores. Each core computes QKV projection for its 4 heads,
RoPE, causal SDPA, and a partial output projection (row-sharded W_out); the
host sums the 4 TP partials per batch element.

Single software-pipelined instruction stream (fp16 matmuls, fp32 PSUM):
  - warmup matmuls on a memset tile ramp the PE p-state while the first
    DMAs land.
  - segment g in 0..3 runs the QKV projection for token slab g (QT/KT
    transposed + RoPE'd on eviction, V natural) interleaved with SDPA for
    query tile g-1; causality means tile g-1 only needs slabs <= g-1, and
    the dependency-free projection matmuls keep the PE busy while ACT/DVE
    drain the softmax chain.
  - SDPA diagonal blocks are column-restricted (queries below the diagonal
    are never computed), the softmax denominator tree runs in fp16 (DVE 4x
    mode) + a ones-matmul partition reduction, and the output projection
    for query tiles 0..2 interleaves into the tq=3 SDPA round.
"""

import sys
from contextlib import ExitStack

sys.path.insert(0, "/opt/trn_rl_repo")

import numpy as np

import bass_rust
import concourse.bacc as bacc
import concourse.mybir as mybir
from concourse.alu_op_type import AluOpType
from concourse import tile
from concourse import bass_utils

B, T, E = 2, 2048, 2048
N_HEAD = 16
D = E // N_HEAD            # 128
THETA = 10000.0
N_CORES = 8
TP = 4                     # tensor-parallel degree (heads)
HPC = N_HEAD // TP         # heads per core = 4
FL = HPC * D               # local head width = 512
EC = E // 128              # 16 contraction chunks
TQ = 512                   # query tile (free dim)
NTQ = T // TQ              # 4
NTK = T // 128             # 16
N_WARM = 8                 # PE p-state warmup matmuls

F32 = mybir.dt.float32
F32R = mybir.dt.float32r
F16 = mybir.dt.float16
EXP = mybir.ActivationFunctionType.Exp
SCALE = 1.0 / np.sqrt(D)

_compiled = None
_last_in_maps = None


def _build():
    nc = bacc.Bacc("TRN2", target_bir_lowering=False)

    xT = nc.dram_tensor("xT", (NTQ, 128, EC * TQ), F16, kind="ExternalInput")
    wqk = nc.dram_tensor("wqk", (2 * HPC, 128, EC * 128), F16, kind="ExternalInput")
    wv = nc.dram_tensor("wv", (128, EC * FL), F16, kind="ExternalInput")
    wout = nc.dram_tensor("wout", (HPC, 128, E), F16, kind="ExternalInput")
    csx = nc.dram_tensor("csx", (128, T), F16, kind="ExternalInput")
    csx2 = nc.dram_tensor("csx2", (128, T), F16, kind="ExternalInput")
    mask1 = nc.dram_tensor("mask1", (128, 128), F16, kind="ExternalInput")
    ones_m = nc.dram_tensor("ones_m", (128, 128), F16, kind="ExternalInput")
    out = nc.dram_tensor("out", (T, E), F16, kind="ExternalOutput")

    with tile.TileContext(nc) as tc, nc.allow_low_precision(
        reason="fp16 matmul inputs / fp16 softmax stats are intentional"
    ), tc.tile_pool(name="const", bufs=1) as const, \
         tc.tile_pool(name="wo_p", bufs=1) as wo_p, \
         tc.tile_pool(name="qkt_p", bufs=1) as qkt_p, \
         tc.tile_pool(name="v_p", bufs=1) as v_p, \
         tc.tile_pool(name="yt_p", bufs=1) as yt_p, \
         tc.tile_pool(name="es_p", bufs=10) as es_p, \
         tc.tile_pool(name="pair_p", bufs=4) as pair_p, \
         tc.tile_pool(name="dn_p", bufs=2) as dn_p, \
         tc.tile_pool(name="mm_ps", bufs=2, space="PSUM") as mm_ps, \
         tc.tile_pool(name="sc_ps", bufs=3, space="PSUM") as sc_ps, \
         tc.tile_pool(name="y_ps", bufs=2, space="PSUM") as y_ps, \
         tc.tile_pool(name="aux_ps", bufs=1, space="PSUM") as aux_ps:

        # phase-1 pools: closed before the final projection segment to free
        # SBUF for the output-eviction pool (stack allocation is LIFO)
        p1_stack = ExitStack()
        wqk_p = p1_stack.enter_context(tc.tile_pool(name="wqk_p", bufs=1))
        wv_p = p1_stack.enter_context(tc.tile_pool(name="wv_p", bufs=1))
        xt_p = p1_stack.enter_context(tc.tile_pool(name="xt_p", bufs=2))
        rope_t = p1_stack.enter_context(tc.tile_pool(name="rope_t", bufs=2))

        cs_sb = const.tile([128, T], F16, tag="cs")    # [cos; sin]
        csd_sb = const.tile([128, T], F16, tag="csd")  # [sin; cos]
        mask_sb = const.tile([128, 128], F16, tag="mask")
        onem = const.tile([128, 128], F16, tag="onem")
        warm_sb = const.tile([128, TQ], F16, tag="warm")

        # wqk stored f8-major: one [128, EC*128] tile per 128-wide qk block so
        # the first projection unit is gated by 0.5MB of weights, not 4MB
        wqk_sb = [wqk_p.tile([128, EC * 128], F16, tag=f"wqk{f}", name=f"wqk_sb{f}")
                  for f in range(2 * HPC)]
        wv_all = wv_p.tile([128, EC * FL], F16, tag="wv")
        wo_sb = [wo_p.tile([128, E], F16, tag=f"wo{h}", name=f"wo_sb{h}")
                 for h in range(HPC)]

        # resident intermediates: QT/KT (transposed, de-interleaved, RoPE'd),
        # V (natural layout), normalized attention outputs
        qkt_sb = [[qkt_p.tile([128, TQ], F16, tag=f"qkt{f}_{t}", name=f"qkt_sb{f}_{t}")
                   for t in range(NTQ)] for f in range(2 * HPC)]
        v_sb = [v_p.tile([128, FL], F16, tag=f"v{t}", name=f"v_sb{t}")
                for t in range(NTK)]
        yt_sb = [[yt_p.tile([128, TQ], F16, tag=f"yt{h}_{t}", name=f"yt_sb{h}_{t}")
                  for t in range(NTQ)] for h in range(HPC)]

        # ---------------- DMA + warmup ----------------
        xt_sb = {}

        def dma_slab(t4):
            xt = xt_p.tile([128, EC * TQ], F16, tag="xt", name=f"xt_{t4}")
            nc.sync.dma_start(xt[:], xT[t4])
            xt_sb[t4] = xt

        # PE p-state warmup on a memset tile (no DMA dependency)
        nc.vector.memset(warm_sb[:], 1.0)

        # startup DMA: slab 0 split into per-chunk column DMAs in consumption
        # order on one queue so the first QK unit streams as chunks land
        # (subtile deps); weights flow in parallel on the other two queues
        xt0 = xt_p.tile([128, EC * TQ], F16, tag="xt", name="xt_0")
        xt_sb[0] = xt0

        for e in range(EC):
            nc.sync.dma_start(xt0[:, e * TQ:(e + 1) * TQ],
                              xT[0, :, e * TQ:(e + 1) * TQ])
        nc.sync.dma_start(cs_sb[:], csx[:])
        nc.sync.dma_start(csd_sb[:], csx2[:])
        for f in range(2 * HPC):
            nc.scalar.dma_start(wqk_sb[f][:], wqk[f])
        nc.gpsimd.dma_start(wv_all[:], wv[:])
        nc.gpsimd.dma_start(mask_sb[:], mask1[:])
        nc.gpsimd.dma_start(onem[:], ones_m[:])
        for h in range(HPC):
            nc.scalar.dma_start(wo_sb[h][:], wout[h])

        wps = aux_ps.tile([128, TQ], F32, tag="aux", name="warm_ps")
        for i in range(N_WARM):
            nc.tensor.matmul(wps[:], warm_sb[:, 0:128], warm_sb[:],
                             start=True, stop=True, skip_group_check=True)

        # ---------------- unit builders ----------------
        def qk_unit(t4, f8):
            def f():
                ps = mm_ps.tile([128, TQ], F32, tag="mm", name=f"qkps_{t4}_{f8}")
                for e in range(EC):
                    nc.tensor.matmul(
                        ps[:], wqk_sb[f8][:, e * 128:(e + 1) * 128],
                        xt_sb[t4][:, e * TQ:(e + 1) * TQ],
                        start=(e == 0), stop=(e == EC - 1),
                        skip_group_check=True,
                    )
                ts4 = slice(t4 * TQ, (t4 + 1) * TQ)
                dst = qkt_sb[f8][t4]
                qk16 = rope_t.tile([128, TQ], F16, tag="qk16")
                nc.scalar.copy(qk16[:], ps[:])
                t_a = rope_t.tile([64, TQ], F16, tag="ta")
                t_b = rope_t.tile([64, TQ], F16, tag="tb")
                nc.vector.tensor_tensor(t_a[:], qk16[0:64, :], cs_sb[0:64, ts4], op=AluOpType.mult)
                nc.vector.tensor_tensor(t_b[:], qk16[64:128, :], cs_sb[64:128, ts4], op=AluOpType.mult)
                nc.vector.tensor_tensor(dst[0:64, :], t_a[:], t_b[:], op=AluOpType.subtract)
                t_c = rope_t.tile([64, TQ], F16, tag="tc")
                t_d = rope_t.tile([64, TQ], F16, tag="td")
                nc.vector.tensor_tensor(t_c[:], qk16[0:64, :], csd_sb[0:64, ts4], op=AluOpType.mult)
                nc.vector.tensor_tensor(t_d[:], qk16[64:128, :], csd_sb[64:128, ts4], op=AluOpType.mult)
                nc.vector.tensor_tensor(dst[64:128, :], t_c[:], t_d[:], op=AluOpType.add)
            return (16 * 216, f)

        def v_unit(t4, i):
            def f():
                tk = 4 * t4 + i
                ps = mm_ps.tile([128, FL], F32, tag="mm", name=f"vps_{tk}")
                for e in range(EC):
                    nc.tensor.matmul(
                        ps[:], xt_sb[t4][:, e * TQ + i * 128:e * TQ + (i + 1) * 128],
                        wv_all[:, e * FL:(e + 1) * FL],
                        start=(e == 0), stop=(e == EC - 1),
                        skip_group_check=True,
                    )
                nc.scalar.copy(v_sb[tk][:], ps[:])
            return (16 * 216, f)

        def p1_units(t4):
            return [qk_unit(t4, f8) for f8 in range(2 * HPC)] + \
                   [v_unit(t4, i) for i in range(4)]

        def sdpa_units(tq):
            units = []
            head_units = []
            for h in range(HPC):
                st = {"fulls": [], "rag": [], "yps": None, "es": {}}
                nblk = 4 * tq + 4

                def blk_a(h, tq, tk, st):
                    def f():
                        r = tk - 4 * tq
                        diag = r >= 0
                        c0 = r * 128 if diag else 0
                        cr = slice(c0, TQ)
                        sps = sc_ps.tile([128, TQ], F32, tag="sc",
                                         name=f"sps_{h}_{tq}_{tk}")
                        nc.tensor.matmul(
                            sps[:, cr],
                            qkt_sb[HPC + h][tk // 4][:, (tk % 4) * 128:(tk % 4 + 1) * 128],
                            qkt_sb[h][tq][:, cr],
                            start=True, stop=True, skip_group_check=True,
                        )
                        es = es_p.tile([128, TQ], F16, tag="es",
                                       name=f"es_{h}_{tq}_{tk}")
                        nc.scalar.activation(es[:, cr], sps[:, cr], EXP, scale=SCALE)
                        if diag:
                            nc.vector.tensor_tensor(
                                es[:, c0:c0 + 128], es[:, c0:c0 + 128],
                                mask_sb[:], op=AluOpType.mult,
                            )
                        st["es"][tk] = es
                    return (int(216 * (TQ - (max(tk - 4 * tq, 0)) * 128) / TQ), f)

                def blk_b(h, tq, tk, st):
                    def f():
                        r = tk - 4 * tq
                        diag = r >= 0
                        c0 = r * 128 if diag else 0
                        cr = slice(c0, TQ)
                        if tk == 0:
                            st["yps"] = y_ps.tile([128, TQ], F32, tag="y",
                                                  name=f"yps_{h}_{tq}")
                        es = st["es"][tk]
                        nc.tensor.matmul(
                            st["yps"][:, cr], v_sb[tk][:, h * 128:(h + 1) * 128],
                            es[:, cr],
                            start=(tk == 0), stop=(tk == 4 * tq + 3),
                            skip_group_check=True,
                        )
                        # denominator bookkeeping: full-width tiles pair up in
                        # fp16 (DVE 4x); ragged diagonals accumulate at the end
                        if not diag or r == 0:
                            st["fulls"].append(es)
                            if len(st["fulls"]) >= 2:
                                a = st["fulls"].pop(0)
                                b = st["fulls"].pop(0)
                                pr = pair_p.tile([128, TQ], F16, tag="pr",
                                                 name=f"pr_{h}_{tq}_{tk}")
                                nc.vector.tensor_tensor(pr[:], a[:], b[:],
                                                        op=AluOpType.add)
                                st["fulls"].append(pr)
                        else:
                            st["rag"].append((es, c0))
                    return (int(216 * (TQ - (max(tk - 4 * tq, 0)) * 128) / TQ), f)

                def denom(h, tq, st):
                    def f():
                        dacc = st["fulls"][0]
                        for es, c0 in st["rag"]:
                            nc.vector.tensor_tensor(
                                dacc[:, c0:], dacc[:, c0:], es[:, c0:],
                                op=AluOpType.add,
                            )
                        # ones[128,128] @ dacc = column sums replicated on all
                        # partitions: reduction + broadcast in one matmul
                        dbc = aux_ps.tile([128, TQ], F32, tag="aux",
                                          name=f"dbc_{h}_{tq}")
                        nc.tensor.matmul(dbc[:], onem[:], dacc[:],
                                         start=True, stop=True,
                                         skip_group_check=True)
                        rcp = dn_p.tile([128, TQ], F32, tag="rcp",
                                        name=f"rcp_{h}_{tq}")
                        nc.vector.reciprocal_approx_fast(rcp[:], dbc[:])
                        nc.vector.tensor_tensor(
                            yt_sb[h][tq][:], st["yps"][:], rcp[:],
                            op=AluOpType.mult,
                        )
                    return (300, f)

                # scores (A) and AV (B) as separate units, software-pipelined
                # at depth 2 so the PE never sits directly behind the exp
                A = [blk_a(h, tq, tk, st) for tk in range(nblk)]
                Bv = [blk_b(h, tq, tk, st) for tk in range(nblk)]
                depth = 2
                stream = []
                for k in range(nblk + depth):
                    if k < nblk:
                        stream.append(A[k])
                    if k >= depth:
                        stream.append(Bv[k - depth])
                head_units.append((stream, denom(h, tq, st)))
            for h in range(HPC):
                blocks, dn = head_units[h]
                k = min(2, len(blocks))
                units.extend(blocks[:k])
                if h > 0:
                    units.append(head_units[h - 1][1])
                units.extend(blocks[k:])
            units.append(head_units[HPC - 1][1])
            return units

        pools = {}

        def proj_unit(tqb, nb):
            def f():
                ps = mm_ps.tile([128, TQ], F32, tag="mm", name=f"ops_{tqb}_{nb}")
                for h in range(HPC):
                    nc.tensor.matmul(
                        ps[:],
                        yt_sb[h][tqb // 4][:, (tqb % 4) * 128:(tqb % 4 + 1) * 128],
                        wo_sb[h][:, nb * TQ:(nb + 1) * TQ],
                        start=(h == 0), stop=(h == HPC - 1),
                        skip_group_check=True,
                    )
                osb = pools["o_ev"].tile([128, TQ], F16, tag="osb",
                                         name=f"osb_{tqb}_{nb}")
                if nb % 2 == 0:
                    nc.vector.tensor_copy(osb[:], ps[:])
                else:
                    nc.scalar.copy(osb[:], ps[:])
                nc.sync.dma_start(
                    out[tqb * 128:(tqb + 1) * 128, nb * TQ:(nb + 1) * TQ], osb[:]
                )
            return (4 * 216, f)

        # ---------------- weave + emit ----------------
        def weave(a, b, bias=2000):
            # a = filler stream (phase-1/proj), b = latency-sensitive stream
            # (sdpa); bias keeps b ahead so a covers the segment tail
            ta = sum(w for w, _ in a) or 1
            tb = sum(w for w, _ in b) or 1
            ca = cb = 0
            i = j = 0
            while i < len(a) or j < len(b):
                if j >= len(b) or (i < len(a) and ca * tb <= max(cb - bias, 0) * ta):
                    ca += a[i][0]
                    a[i][1]()
                    i += 1
                else:
                    cb += b[j][0]
                    b[j][1]()
                    j += 1

        for _, f in p1_units(0):
            f()
        for g in range(1, NTQ):
            dma_slab(g)
            weave(p1_units(g), sdpa_units(g - 1))
        p1_stack.close()
        with tc.tile_pool(name="o_ev", bufs=4) as o_ev:
            pools["o_ev"] = o_ev
            proj = [proj_unit(tqb, nb) for tqb in range(12) for nb in range(4)]
            weave(proj[:36], sdpa_units(NTQ - 1))
            for _, f in proj[36:]:
                f()
            for tqb in range(12, 16):
                for nb in range(4):
                    proj_unit(tqb, nb)[1]()

    nc.compile()
    return nc


def _host_tables():
    positions = np.arange(T, dtype=np.float64)
    inv_freq = 1.0 / (THETA ** (np.arange(0, D, 2, dtype=np.float64) / D))
    freqs = np.outer(positions, inv_freq)          # [T, 64]
    cs = np.concatenate([np.cos(freqs).T, np.sin(freqs).T]).astype(np.float16)   # [128, T]
    cs2 = np.concatenate([np.sin(freqs).T, np.cos(freqs).T]).astype(np.float16)  # swapped halves
    p = np.arange(128)[:, None]
    j = np.arange(128)[None, :]
    mask = (p <= j).astype(np.float16)             # [128, 128] lower-tri visibility
    return cs, cs2, mask


def kernel(x, W_qkv, W_out):
    global _compiled
    if _compiled is None:
        _compiled = _build()
    nc = _compiled

    x = np.ascontiguousarray(np.asarray(x, dtype=np.float32))
    W_qkv = np.asarray(W_qkv, dtype=np.float32)
    W_out = np.asarray(W_out, dtype=np.float32)

    cs, cs2, mask = _host_tables()
    ones_mm = np.ones((128, 128), np.float16)

    perm = np.concatenate([np.arange(0, D, 2), np.arange(1, D, 2)])  # de-interleave

    in_maps = []
    for c in range(N_CORES):
        b, tp = divmod(c, TP)
        heads = np.arange(tp * HPC, (tp + 1) * HPC)
        qk_cols = np.concatenate(
            [h * D + perm for h in heads] + [E + h * D + perm for h in heads]
        )
        v_cols = np.concatenate([2 * E + h * D + np.arange(D) for h in heads])
        # wqk f8-major: [f8, p, e*128+m]
        wqk_l = np.ascontiguousarray(
            W_qkv[:, qk_cols].reshape(EC, 128, 2 * HPC, 128)
            .transpose(2, 1, 0, 3).reshape(2 * HPC, 128, EC * 128)
        )
        # wv e-major along free dim: [p, e*FL+m]
        wv_l = np.ascontiguousarray(
            W_qkv[:, v_cols].reshape(EC, 128, FL)
            .transpose(1, 0, 2).reshape(128, EC * FL)
        )
        wout_l = np.ascontiguousarray(
            W_out.reshape(N_HEAD, D, E)[heads].reshape(HPC, 128, E)
        )
        # x slab: [t4, p, e*TQ+t]
        xt4 = np.ascontiguousarray(
            x[b].reshape(NTQ, TQ, EC, 128).transpose(0, 3, 2, 1)
            .reshape(NTQ, 128, EC * TQ)
        ).astype(np.float16)
        in_maps.append({
            "xT": xt4,
            "wqk": wqk_l.astype(np.float16),
            "wv": wv_l.astype(np.float16),
            "wout": wout_l.astype(np.float16),
            "csx": cs,
            "csx2": cs2,
            "mask1": mask,
            "ones_m": ones_mm,
        })

    global _last_in_maps
    _last_in_maps = in_maps
    res = bass_utils.run_bass_kernel_spmd(nc, in_maps, core_ids=list(range(N_CORES)))
    out = np.zeros((B, T, E), dtype=np.float32)
    for c in range(N_CORES):
        out[c // TP] += res.results[c]["out"]
    return out
